# revision 50
# baseline (speedup 1.0000x reference)
"""Trainium2 Bass kernel for multi-head self-attention with RoPE (causal).

Problem (hardcoded): B=4, S=2048, D=1024, H=16 heads, d_k=64, fp32 I/O.

Sharding over 8 NeuronCores: core c handles batch b=c//2 and head-group
g=c%2 (8 heads each).  Q/K/V/O projections are head-sliced (tensor
parallel); each core computes a partial output (its 8 heads) through the
row-sharded Wo, and the host sums the two partials per batch.

Per-core kernel v3 (all matmuls on PE, fp16 operands, fp32 PSUM):
  - v projection appends a ones-column per head (65-col pitch) so the
    PV matmul simultaneously accumulates the softmax denominator L in
    psum row 64 -- no separate ones-matmul stream.
  - attention blocked [kj=128] x [qi=512], one head PAIR per block;
    j order per (pair, chunk): diagonal g0 first (full-width psum
    start), off-diagonals next (big exp blocks keep the ACT engine fed
    across the pair boundary), column-trimmed diagonals g1-g3 last
    (cheap PV tail; their psum stop covers a partial region, which is
    sim bookkeeping only); causal triangle via one [128,128] fp16
    multiplicative mask on the leading 128 columns of each diagonal.
  - PV emission is software-pipelined two blocks behind the score
    matmuls so the PE never waits for the ACT exp of the same block.
  - per (pair, qi-chunk) finalize, split in two stages deferred into
    the next pair's blocks: (a) drain pv psum to sbuf (frees the pv
    bank) + full-tile DVE fast-reciprocal (custom-DVE ops are only
    correct at partition base 0 on hw); (b) broadcast 1/L over the 64
    dv rows with a K=128 fp16 selector matmul into a score-pool slot,
    DVE-multiply, and merge the head-1 half into the outs tile by an
    sbuf->sbuf DMA (DVE operands must stay partition-aligned).
  - PSUM: score pool 3x[128,1024] + pv 2x[65,512] = 8 banks; the
    third score buffer absorbs the exp-completion semaphore latency
    on the slot recycle; v/qk/Wo projections and the 1/L broadcasts
    reuse score slots.
  - loops are qi-major; Wo matmuls of chunk i-1 are drip-fed one per
    block into the ACT-bound attention of chunk i (exp is the
    per-block bottleneck, so the PE has idle slots to fill); output
    is fp16 (host sums the two partials per batch in fp32).
"""

import math

import numpy as np

B, S, D, H = 4, 2048, 1024, 16
WO_DRIP = True
DK = 64          # head dim
HPC = 8          # heads per core
QI = 512         # qi chunk width
KJ = 128         # kj chunk width
N_CORES = 8
THETA = 10000.0

F16 = "float16"


# ---------------------------------------------------------------------------
# Bass program (same NEFF for all cores; per-core data differs)
# ---------------------------------------------------------------------------

def build_nc(seq_len=S, debug=False, taps=False):
    import concourse.bass as bass
    import concourse.mybir as mybir
    import concourse.tile as tile
    from concourse import bacc

    fp16 = mybir.dt.float16
    fp32 = mybir.dt.float32
    AF = mybir.ActivationFunctionType
    MUL = mybir.AluOpType.mult

    s = seq_len
    n_stile = s // 128         # 128-row s-chunks (16)
    n_qi = s // QI             # qi chunks (4)
    n_dt = D // 128            # d (contraction) tiles = 8

    kwargs = dict(target_bir_lowering=False, debug=True) if debug else {}
    nc = bacc.Bacc("TRN2", **kwargs)

    xT = nc.declare_dram_parameter("xT", [D, s], fp16, isOutput=False)
    wqT = nc.declare_dram_parameter("wqT", [D, 512], fp16, isOutput=False)
    wkT = nc.declare_dram_parameter("wkT", [D, 512], fp16, isOutput=False)
    wvT = nc.declare_dram_parameter("wvT", [D, 512], fp16, isOutput=False)
    woT = nc.declare_dram_parameter("woT", [512, D], fp16, isOutput=False)
    cosT = nc.declare_dram_parameter("cosT", [128, s], fp16, isOutput=False)
    sinT = nc.declare_dram_parameter("sinT", [128, s], fp16, isOutput=False)
    # triangle mask: tri[p, c] = 1 if c >= p else 0
    tri = nc.declare_dram_parameter("tri", [128, 128], fp16, isOutput=False)
    out = nc.declare_dram_parameter("out", [s, D], fp16, isOutput=True)
    if taps:
        tap_v = nc.declare_dram_parameter("tap_v", [128, 520], fp16, isOutput=True)
        tap_q = nc.declare_dram_parameter("tap_q", [128, s], fp16, isOutput=True)
        tap_k = nc.declare_dram_parameter("tap_k", [128, s], fp16, isOutput=True)
        tap_et = nc.declare_dram_parameter("tap_et", [128, 1024], fp16, isOutput=True)
        tap_pc = nc.declare_dram_parameter("tap_pc", [65, 512], fp32, isOutput=True)
        tap_bc = nc.declare_dram_parameter("tap_bc", [64, 512], fp32, isOutput=True)
        tap_o = nc.declare_dram_parameter("tap_o", [512, s], fp16, isOutput=True)

    with tile.TileContext(nc) as tc:
        with (
            tc.tile_pool(name="const", bufs=1) as const,
            tc.tile_pool(name="qk", bufs=1) as qkp,
            tc.tile_pool(name="ropetmp", bufs=2) as ropetmp,
            tc.tile_pool(name="expp", bufs=4) as expp,
            tc.tile_pool(name="outs", bufs=1) as outsp,
            tc.tile_pool(name="small", bufs=2) as small,
            tc.tile_pool(name="bcp", bufs=2) as bcp,
            tc.tile_pool(name="fsp", bufs=2) as fsp,
            tc.tile_pool(name="sc", bufs=3, space="PSUM") as scp,
            tc.tile_pool(name="pv", bufs=1, space="PSUM") as pvp,
        ):
            # ---- inputs to SBUF; xT split into column halves so the
            # first half of the projections can start ~6us earlier ----
            h2 = s // 2
            q4 = s // 4
            xT_sb = [const.tile([128, s], fp16, tag=f"xT{d}", name=f"xT{d}")
                     for d in range(n_dt)]
            w_sb = {"v": []}
            for d in range(n_dt):
                nc.sync.dma_start(out=xT_sb[d][:, 0:q4],
                                  in_=xT[d * 128:(d + 1) * 128, 0:q4])
                t = const.tile([128, 512], fp16, tag=f"wv{d}", name=f"wv{d}")
                nc.sync.dma_start(out=t[:, :],
                                  in_=wvT[d * 128:(d + 1) * 128, :])
                w_sb["v"].append(t)
            for d in range(n_dt):
                nc.sync.dma_start(out=xT_sb[d][:, q4:h2],
                                  in_=xT[d * 128:(d + 1) * 128, q4:h2])
            for d in range(n_dt):
                nc.sync.dma_start(out=xT_sb[d][:, h2:s],
                                  in_=xT[d * 128:(d + 1) * 128, h2:s])
            for name, dram in (("q", wqT), ("k", wkT)):
                tiles = []
                for d in range(n_dt):
                    t = const.tile([128, 512], fp16, tag=f"w{name}{d}",
                                   name=f"w{name}{d}")
                    nc.sync.dma_start(out=t[:, :],
                                      in_=dram[d * 128:(d + 1) * 128, :])
                    tiles.append(t)
                w_sb[name] = tiles
            cos_sb = const.tile([128, s], fp16, tag="cos")
            nc.sync.dma_start(out=cos_sb[:, :], in_=cosT[:, :])
            sin_sb = const.tile([128, s], fp16, tag="sin")
            nc.sync.dma_start(out=sin_sb[:, :], in_=sinT[:, :])
            tri_sb = const.tile([128, 128], fp16, tag="tri")
            nc.sync.dma_start(out=tri_sb[:, :], in_=tri[:, :])
            wo_sb = []
            for o in range(4):
                t = const.tile([128, D], fp16, tag=f"wo{o}", name=f"wo{o}")
                nc.sync.dma_start(out=t[:, :], in_=woT[o * 128:(o + 1) * 128, :])
                wo_sb.append(t)

            # selector for the 1/L broadcast matmul: K=128 fp16 with ones
            # in row 64 (same geometry as a plain full-height matmul)
            sel64 = const.tile([128, 64], fp16, tag="sel64")
            nc.vector.memset(sel64[:, :], 0.0)
            nc.vector.memset(sel64[64:65, :], 1.0)
            # persistent 1/L staging rows (zeroed once; only row 64 written)
            rl16A_t = const.tile([128, QI], fp16, tag="rl16A")
            nc.vector.memset(rl16A_t[:, :], 0.0)
            rl16B_t = const.tile([128, QI], fp16, tag="rl16B")
            nc.vector.memset(rl16B_t[:, :], 0.0)

            # ---- v projection into 65-col-pitch tiles (ones col at 64) ----
            v_sb = []
            for st in range(n_stile):
                vt = const.tile([128, 8 * 65], fp16, tag=f"v{st}", name=f"v{st}")
                nc.gpsimd.memset(vt[:, :], 1.0)
                ps = scp.tile([128, 1024], fp32, tag="sc", name="psv")
                for d in range(n_dt):
                    nc.tensor.matmul(ps[:, 0:512],
                                     lhsT=xT_sb[d][:, st * 128:(st + 1) * 128],
                                     rhs=w_sb["v"][d][:, :],
                                     start=(d == 0), stop=(d == n_dt - 1))
                dst = vt[:, :].rearrange("p (h c) -> p h c", c=65)[:, :, 0:64]
                src = ps[:, 0:512].rearrange("p (h c) -> p h c", c=64)
                nc.vector.tensor_copy(dst, src)
                if taps and st == 0:
                    nc.sync.dma_start(out=tap_v[:, :], in_=vt[:, :])
                v_sb.append(vt)

            outs_sb = [outsp.tile([128, s], fp16, tag=f"outs{p}",
                                  name=f"outs{p}") for p in range(4)]

            # ---- q/k projections + rope, per head pair ----
            qT_all = {}
            kT_all = {}
            for pair in range(4):
                oc = pair * 128  # o-column offset of this pair in [0,512)
                qt = qkp.tile([128, s], fp16, tag=f"qT{pair}", name="qt")
                kt = qkp.tile([128, s], fp16, tag=f"kT{pair}", name="kt")
                for name, dst in (("q", qt), ("k", kt)):
                    for n in range(s // 1024):
                        ps = scp.tile([128, 1024], fp32, tag="sc", name="psqk")
                        for half in range(2):
                            c0 = n * 1024 + half * 512
                            for d in range(n_dt):
                                nc.tensor.matmul(
                                    ps[:, half * 512:half * 512 + 512],
                                    lhsT=w_sb[name][d][:, oc:oc + 128],
                                    rhs=xT_sb[d][:, c0:c0 + 512],
                                    start=(d == 0), stop=(d == n_dt - 1))
                        nc.vector.tensor_copy(dst[:, n * 1024:(n + 1) * 1024],
                                              ps[:, :])
                # rope: swapped copy via 4 block DMAs, then 3 DVE ops
                for name, t in (("q", qt), ("k", kt)):
                    sw = ropetmp.tile([128, s], fp16, tag="swap", name="sw")
                    for blk in range(4):
                        a, bb = blk * 32, (blk ^ 1) * 32
                        nc.sync.dma_start(out=sw[a:a + 32, :],
                                          in_=t[bb:bb + 32, :])
                    tmp = ropetmp.tile([128, s], fp16, tag="ropetmp",
                                       name="rtmp")
                    nc.vector.tensor_tensor(tmp[:, :], t[:, :], cos_sb[:, :],
                                            op=MUL)
                    nc.vector.tensor_tensor(sw[:, :], sw[:, :], sin_sb[:, :],
                                            op=MUL)
                    nc.vector.tensor_tensor(t[:, :], tmp[:, :], sw[:, :],
                                            op=mybir.AluOpType.add)
                if taps and pair == 0:
                    nc.sync.dma_start(out=tap_q[:, :], in_=qt[:, :])
                    nc.sync.dma_start(out=tap_k[:, :], in_=kt[:, :])
                qT_all[pair] = qt
                kT_all[pair] = kt

            # ---- attention, qi-major so Wo(i) overlaps attention(i+1) ----
            # per-i block list: (ks, qoff, mask_off, zero_to) with diagonal
            # blocks first -- the j==0 start and last-block stop are always
            # full width.
            def blocks(i):
                # diag g0 first (full-width start), off-diagonals next (big
                # exp blocks keep ACT fed across the pair boundary), trimmed
                # diagonals last (cheap PV tail; the stop lands on a partial
                # region, which is sim bookkeeping only)
                bl = [(512 * i, 0, 0, 0)]
                for j in range(4 * i):
                    bl.append((128 * j, 0, None, 0))
                for g in range(1, 4):
                    bl.append((512 * i + 128 * g, 128 * g, 0, 0))
                return bl

            def wo_gen(i, pool=None):
                """Wo(chunk i) as a generator: one PE matmul per step so it
                can be drip-fed into the ACT-bound attention blocks."""
                pool = pool or scp
                for sti in range(4):
                    st = 4 * i + sti
                    ps = pool.tile([128, 1024], fp32, tag="sc", name="pswo")
                    for half in range(2):
                        for o in range(4):
                            nc.tensor.matmul(
                                ps[:, half * 512:half * 512 + 512],
                                lhsT=outs_sb[o][:, st * 128:(st + 1) * 128],
                                rhs=wo_sb[o][:, half * 512:half * 512 + 512],
                                start=(o == 0), stop=(o == 3))
                            yield
                    fs = fsp.tile([128, 1024], fp16, tag="fs", name="fs")
                    nc.vector.tensor_copy(fs[:, :], ps[:, :])
                    nc.sync.dma_start(out=out[st * 128:(st + 1) * 128, :],
                                      in_=fs[:, :])

            f32r = mybir.dt.float32r

            def make_fin(pv0, pv1, pair, qs):
                state = {}

                def fin_a():
                    # drain pv psum to sbuf -- frees the pv bank for the
                    # next pair -- and take reciprocals of the L row
                    pcA = small.tile([65, QI], fp32, tag="pcA", name="pcA")
                    pcB = small.tile([65, QI], fp32, tag="pcB", name="pcB")
                    nc.vector.tensor_copy(pcA[:, :], pv0[:, :])
                    nc.vector.tensor_copy(pcB[:, :], pv1[:, :])
                    rlA = small.tile([65, QI], fp32, tag="rlA", name="rlA")
                    rlB = small.tile([65, QI], fp32, tag="rlB", name="rlB")
                    # custom-DVE ops are only reliable at partition base 0
                    # on hw: run the reciprocal over the whole tile and use
                    # row 64 (1/L); rows 0-63 (1/pv junk) are never read
                    nc.vector.reciprocal_approx_fast(out=rlA[0:65, :],
                                                     in_=pcA[0:65, :])
                    nc.vector.reciprocal_approx_fast(out=rlB[0:65, :],
                                                     in_=pcB[0:65, :])
                    nc.vector.tensor_copy(rl16A_t[64:65, 0:QI],
                                          rlA[64:65, :])
                    nc.vector.tensor_copy(rl16B_t[64:65, 0:QI],
                                          rlB[64:65, :])
                    state.update(pcA=pcA, pcB=pcB, rlA=rl16A_t,
                                 rlB=rl16B_t)

                def fin_b():
                    # broadcast 1/L over the 64 dv partitions with K=1
                    # f32r matmuls (psum from the sc pool), then scale;
                    # head-1 is staged and merged by an sbuf->sbuf DMA
                    # (DVE operands stay partition-aligned)
                    pcA, pcB = state["pcA"], state["pcB"]
                    bcA = scp.tile([128, 1024], fp32, tag="sc", name="bcA")
                    nc.tensor.matmul(bcA[0:64, 0:QI], lhsT=sel64[:, :],
                                     rhs=state["rlA"][:, 0:QI],
                                     start=True, stop=True)
                    bcB = scp.tile([128, 1024], fp32, tag="sc", name="bcB")
                    nc.tensor.matmul(bcB[0:64, 0:QI], lhsT=sel64[:, :],
                                     rhs=state["rlB"][:, 0:QI],
                                     start=True, stop=True)
                    if taps and pair == 0 and qs == 0:
                        nc.sync.dma_start(out=tap_pc[:, :], in_=pcA[:, :])
                        bst = small.tile([64, QI], fp32, tag="bst",
                                         name="bst")
                        nc.vector.tensor_copy(bst[:, :], bcA[0:64, 0:QI])
                        nc.sync.dma_start(out=tap_bc[:, :], in_=bst[:, :])
                    nc.vector.tensor_tensor(outs_sb[pair][0:64, qs:qs + QI],
                                            pcA[0:64, :], bcA[0:64, 0:QI],
                                            op=MUL)
                    o1 = small.tile([64, QI], fp16, tag="o1", name="o1")
                    nc.vector.tensor_tensor(o1[:, :], pcB[0:64, :],
                                            bcB[0:64, 0:QI], op=MUL)
                    nc.sync.dma_start(out=outs_sb[pair][64:128, qs:qs + QI],
                                      in_=o1[:, :])
                return fin_a, fin_b

            fin_prev = None
            wo_iter = None
            wo_pace = [0, 1]   # [counter, stride]
            pending_wo = []
            for i in range(n_qi):
                qs = i * QI
                for pair in range(4):
                    qt, kt = qT_all[pair], kT_all[pair]
                    pv0 = pvp.tile([65, QI], fp32, tag="pv0", name="pv0")
                    pv1 = pvp.tile([65, QI], fp32, tag="pv1", name="pv1")
                    bl = blocks(i)
                    nb = len(bl)
                    pend = []

                    def emit_pv(p):
                        et, w, first, last, ks = p
                        vst = v_sb[ks // 128]
                        qoff = QI - w
                        for h, pv in ((0, pv0), (1, pv1)):
                            nc.tensor.matmul(
                                pv[0:65, qoff:QI],
                                lhsT=vst[:, (2 * pair + h) * 65:
                                          (2 * pair + h) * 65 + 65],
                                rhs=et[:, h * w:h * w + w],
                                start=first, stop=last,
                                skip_group_check=True)

                    for idx, (ks, qoff, moff, zto) in enumerate(bl):
                        w = QI - qoff
                        sc = scp.tile([128, 1024], fp32, tag="sc", name="sc")
                        for h in range(2):
                            # head sections at fixed 512 pitch: a matmul
                            # output must stay within one psum bank
                            nc.tensor.matmul(
                                sc[:, h * 512:h * 512 + w],
                                lhsT=kt[64 * h:64 * h + 64, ks:ks + KJ],
                                rhs=qt[64 * h:64 * h + 64, qs + qoff:qs + QI],
                                start=True, stop=True)
                        et = expp.tile([128, 1024], fp16, tag="exp", name="et")
                        if w == 512:
                            nc.scalar.activation(et[:, 0:1024], sc[:, 0:1024],
                                                 AF.Exp, scale=0.125)
                        else:
                            sc3 = sc[:, 0:1024].rearrange(
                                "p (b c) -> p b c", b=2)[:, :, 0:w]
                            et3 = et[:, 0:2 * w].rearrange(
                                "p (b c) -> p b c", b=2)
                            nc.scalar.activation(et3, sc3, AF.Exp,
                                                 scale=0.125)
                        if moff is not None:
                            for h in range(2):
                                base = h * w + moff
                                if zto > 0:
                                    nc.gpsimd.memset(et[:, h * w:base], 0.0)
                                nc.gpsimd.tensor_tensor(
                                    et[:, base:base + 128],
                                    et[:, base:base + 128],
                                    tri_sb[:, :], op=MUL)
                        # PV is emitted two blocks behind so the ACT exp and
                        # the mask are never on the PE critical path
                        if idx == 1 and fin_prev is not None:
                            fin_prev()
                            fin_prev = None
                        if taps and i == 0 and pair == 0 and idx == 0:
                            nc.sync.dma_start(out=tap_et[:, :], in_=et[:, :])
                        pend.append((et, w, idx == 0, idx == nb - 1, ks))
                        if len(pend) > 2:
                            emit_pv(pend.pop(0))
                        # drip Wo matmuls of chunk i-1 into the PE holes
                        # of these ACT-bound blocks, paced to last the chunk
                        if WO_DRIP and wo_iter is not None and (
                                pair > 0 or idx >= 2):
                            wo_pace[0] += 1
                            if wo_pace[0] >= wo_pace[1]:
                                wo_pace[0] = 0
                                if next(wo_iter, "end") == "end":
                                    wo_iter = None
                    for p in pend:
                        emit_pv(p)
                    fin_prev = make_fin(pv0, pv1, pair, qs)
                    if pair == 0 and pending_wo:
                        if wo_iter is not None:
                            for _ in wo_iter:
                                pass
                        wo_iter = wo_gen(pending_wo.pop())
                        wo_pace = [0, 1]
                        if not WO_DRIP:
                            for _ in wo_iter:
                                pass
                            wo_iter = None
                pending_wo.append(i)
            if wo_iter is not None:
                for _ in wo_iter:
                    pass
            fin_prev()
            for _ in wo_gen(pending_wo.pop(), pool=scp):
                pass
            if taps:
                for p in range(4):
                    nc.sync.dma_start(out=tap_o[p * 128:(p + 1) * 128, :],
                                      in_=outs_sb[p][:, :])
    if not debug:
        nc.finalize()
    return nc


# ---------------------------------------------------------------------------
# Host-side input prep
# ---------------------------------------------------------------------------

def _rope_tables(pos, seq_len):
    """cos/sin tables [128, s] fp16 for one batch, rotate-half layout."""
    freqs = (1.0 / (THETA ** (np.arange(0, DK, 2, dtype=np.float32) / DK)))
    ang = pos.astype(np.float32)[None, :] * freqs.astype(np.float32)[:, None]
    c32 = np.cos(ang)  # [32, s]
    s32 = np.sin(ang)
    c64 = np.concatenate([c32, c32], axis=0)          # [64, s]
    s64 = np.concatenate([-s32, s32], axis=0)         # [64, s]
    cos = np.tile(c64, (2, 1)).astype(np.float16)     # [128, s]
    sin = np.tile(s64, (2, 1)).astype(np.float16)
    return cos, sin


def _perm_rows():
    """rotate-half permutation of output dims within each head (evens
    first then odds), for a 512-row weight slice."""
    p = []
    for h in range(HPC):
        base = h * DK
        p.extend(base + np.r_[np.arange(0, DK, 2), np.arange(1, DK, 2)])
    return np.array(p)


def _trimask():
    c = np.arange(128)[None, :]
    p = np.arange(128)[:, None]
    return (c >= p).astype(np.float16)


def make_core_inputs(x, token_positions, Wq, Wk, Wv, Wo, seq_len=S):
    """Returns list of 8 input dicts (one per core)."""
    perm = _perm_rows()
    tri = _trimask()
    ins = []
    for c in range(N_CORES):
        b, g = c // 2, c % 2
        osl = slice(512 * g, 512 * (g + 1))
        wq = Wq[osl][perm]          # [512, D] permuted rows
        wk = Wk[osl][perm]
        wv = Wv[osl]
        cos, sin = _rope_tables(np.asarray(token_positions[b]), seq_len)
        ins.append({
            "xT": np.ascontiguousarray(
                np.asarray(x[b]).T.astype(np.float16)),
            "wqT": np.ascontiguousarray(wq.T.astype(np.float16)),
            "wkT": np.ascontiguousarray(wk.T.astype(np.float16)),
            "wvT": np.ascontiguousarray(wv.T.astype(np.float16)),
            "woT": np.ascontiguousarray(
                Wo[:, osl].T.astype(np.float16)),
            "cosT": cos, "sinT": sin,
            "tri": tri,
        })
    return ins


_NC_CACHE = {}


def kernel(x, token_positions, Wq, Wk, Wv, Wo):
    from concourse.bass_utils import run_bass_kernel_spmd

    x = np.asarray(x)
    if "nc" not in _NC_CACHE:
        _NC_CACHE["nc"] = build_nc(S)
    nc = _NC_CACHE["nc"]
    in_maps = make_core_inputs(x, np.asarray(token_positions),
                               np.asarray(Wq), np.asarray(Wk),
                               np.asarray(Wv), np.asarray(Wo))
    for attempt in range(3):
        res = run_bass_kernel_spmd(nc, in_maps,
                                   core_ids=list(range(N_CORES)))
        outs = [r["out"].astype(np.float32) for r in res.results]
        full = np.stack([outs[2 * b] + outs[2 * b + 1] for b in range(B)])
        if np.isfinite(full).all():
            break
        # transient device glitch (wedged core) -- rerun
    return full
